# revision 40
# baseline (speedup 1.0000x reference)
"""Trainium2 Bass kernel: causal multi-head attention block (QKV proj + RoPE +
causal softmax attention + out proj), distributed over 8 NeuronCores.

Sharding: core = (batch b in 0..3, head-group g in 0..1). Each core computes
the full attention pipeline for its batch and its 16 heads, producing a
partial [T, C] output (its heads' contribution through the out projection).
Host sums the two partials per batch and adds b_proj. No collectives.

Per-core layouts (host-prepared, mostly bf16):
  xt   [128, NTS*KC*512]  x[b].T packed chunk-major: block (ts, k) holds
                          C-rows k*128:(k+1)*128 x tokens ts*512:(ts+1)*512.
                          One DMA per token-chunk -> compute starts after the
                          first 2MB instead of the full 8MB.
  wq   [HP, 128, KC*128]  W_q columns for group g, RoPE-planar-permuted, tiled
  wk   [HP, 128, KC*128]  same for K
  wv   [NVS, 128, KC*VN]  W_v columns for group g, packed per ns-half
  wo   [128, DC*C]        W_proj rows for group g, packed d-major
  cosf/sinf [128, T]      RoPE tables in head-pair row layout (sign folded)
  tri  [128, 128]         causal mask for diagonal blocks (kk <= qq)
  out  [T, C] bf16        partial output

Schedule: phase 1 (V proj, ns-outer so only wv-half0 + xt-chunk0 gate the
first matmul), pairs 0..6 attention with next-pair QKV pumped as background,
then pair 7 attention in a narrower scope (xt/wqk/rope pools closed) with the
out-projection weights prefetched and out-proj matmul groups pumped as its
background work, then the rest of the out projection. PSUM evictions of the
attention accumulators go through the Scalar engine so they are not queued
behind RoPE work on Vector (that stalled the PE at q-block boundaries).
"""

import numpy as np
import ml_dtypes

import concourse.bass as bass
import concourse.bacc as bacc
import concourse.mybir as mybir
import concourse.tile as tile

BF16 = mybir.dt.bfloat16
F32 = mybir.dt.float32
AF = mybir.ActivationFunctionType
NPBF16 = ml_dtypes.bfloat16

N_EMBD = 2048
N_HEAD = 32
HEAD_DIM = 64
B_FULL = 4
T_FULL = 2048
N_CORES = 8
HLOC_FULL = 16  # heads per core


def build_graph(T=2048, C=2048, HLOC=16, qk_bias=False, v_bias=False):
    D = HEAD_DIM
    HP = HLOC // 2          # head pairs per core
    CL = HLOC * D           # local head dims
    KC = C // 128           # contraction chunks for projections
    TT = T // 128           # token tiles
    QBS = min(512, T)       # q-block size
    NQB = T // QBS
    TSW = min(512, T)       # token slice width for qkv psum
    NTS = T // TSW
    VN = min(512, CL)       # v matmul free width
    NVS = CL // VN
    DC = CL // 128          # out-proj contraction chunks
    CS = min(512, C)        # out-proj col slice
    NCS = C // CS
    SCALE = 1.0 / float(np.sqrt(D))

    nc = bacc.Bacc(None, target_bir_lowering=False, debug=False)

    xt_d = nc.declare_dram_parameter("xt", [128, NTS * KC * TSW], BF16, False)
    wq_d = nc.declare_dram_parameter("wq", [HP, 128, KC * 128], BF16, False)
    wk_d = nc.declare_dram_parameter("wk", [HP, 128, KC * 128], BF16, False)
    wv_d = nc.declare_dram_parameter("wv", [NVS, 128, KC * VN], BF16, False)
    wo_d = nc.declare_dram_parameter("wo", [128, DC * C], BF16, False)
    cos_d = nc.declare_dram_parameter("cosf", [128, T], BF16, False)
    sin_d = nc.declare_dram_parameter("sinf", [128, T], BF16, False)
    tri_d = nc.declare_dram_parameter("tri", [128, 128], BF16, False)
    if qk_bias:
        bqk_d = nc.declare_dram_parameter("bqk", [128, 2 * HP], F32, False)
    if v_bias:
        bv_d = nc.declare_dram_parameter("bv", [128, HP], F32, False)
    out_d = nc.declare_dram_parameter("out", [T, C], BF16, True)

    with tile.TileContext(nc) as tc:
        with (
            tc.tile_pool(name="const", bufs=1) as constp,
            tc.tile_pool(name="vall", bufs=1) as vallp,
            tc.tile_pool(name="yt", bufs=1) as ytp,
            tc.tile_pool(name="qkt", bufs=2) as qktp,
            tc.tile_pool(name="esc", bufs=2) as ep,
            tc.tile_pool(name="norm", bufs=1) as normp,
            tc.tile_pool(name="psmm", bufs=2, space="PSUM") as psmm,
            tc.tile_pool(name="pssc", bufs=2, space="PSUM") as pssc,
            tc.tile_pool(name="psyt", bufs=1, space="PSUM") as psyt,
        ):
            # ---- constants ----
            cosf = constp.tile([128, T], BF16, name="cosf", tag="cosf")
            sinf = constp.tile([128, T], BF16, name="sinf", tag="sinf")
            tri = constp.tile([128, 128], BF16, name="tri", tag="tri")
            nc.sync.dma_start(cosf[:], cos_d.ap())
            nc.sync.dma_start(sinf[:], sin_d.ap())
            nc.sync.dma_start(tri[:], tri_d.ap())
            if qk_bias:
                bqk = constp.tile([128, 2 * HP], F32, name="bqk", tag="bqk")
                nc.sync.dma_start(bqk[:], bqk_d.ap())
            if v_bias:
                bv = constp.tile([128, HP], F32, name="bv", tag="bv")
                nc.sync.dma_start(bv[:], bv_d.ap())

            # ---- v_all tiles (65-packed: 64 dims + ones column per head) ----
            vall = []
            for t in range(TT):
                vt = vallp.tile([128, HLOC * 65], BF16, name=f"vall{t}",
                                tag=f"vall{t}")
                vall.append(vt)

            # ---- yT accumulator tiles ----
            ytall = []
            for d_ in range(DC):
                yt_ = ytp.tile([128, T], BF16, name=f"yt{d_}", tag=f"yt{d_}")
                ytall.append(yt_)

            # =============== shared attention helpers ===============
            def geom(kt_i, nfull):
                if kt_i < nfull:
                    return 0, QBS
                i = kt_i - nfull
                return 128 * i, QBS - 128 * i

            def score_pair(qt, kt, pi, nfull, q0):
                """Score matmuls + merged exp + tri-masks for groups
                (2*pi, 2*pi+1) sharing one 4-bank psum quad and one e tile.
                Merging the two groups' exps into one ACT call saves the
                per-call access latency + semaphore wait on the engine that
                paces the attn@V matmuls."""
                sc4 = pssc.tile([128, 4 * QBS], F32, name="sc4", tag="sc",
                                bufs=1)
                e4 = ep.tile([128, 4 * QBS], BF16, name="e4", tag="e",
                             bufs=1)
                gs = []
                for half, kt_i in ((0, 2 * pi), (1, 2 * pi + 1)):
                    off, N = geom(kt_i, nfull)
                    ksl = slice(kt_i * 128, (kt_i + 1) * 128)
                    qsl = slice(q0 + off, q0 + QBS)
                    b = half * 2 * QBS
                    nc.tensor.matmul(sc4[:, b:b + N], kt[0:64, ksl],
                                     qt[0:64, qsl], start=True, stop=True)
                    nc.tensor.matmul(sc4[:, b + QBS:b + QBS + N],
                                     kt[64:128, ksl], qt[64:128, qsl],
                                     start=True, stop=True)
                    gs.append((kt_i, off, N, b))
                Na, Nb = gs[0][2], gs[1][2]
                if Na + Nb > 512:
                    span = 3 * QBS + Nb
                    nc.scalar.activation(e4[:, 0:span], sc4[:, 0:span],
                                         AF.Exp, scale=SCALE)
                else:
                    for (kt_i, off, N, b) in gs:
                        nc.scalar.activation(e4[:, b:b + N], sc4[:, b:b + N],
                                             AF.Exp, scale=SCALE)
                        nc.scalar.activation(e4[:, b + QBS:b + QBS + N],
                                             sc4[:, b + QBS:b + QBS + N],
                                             AF.Exp, scale=SCALE)
                for (kt_i, off, N, b) in gs:
                    if kt_i >= nfull:
                        nc.vector.tensor_mul(e4[:, b:b + 128],
                                             e4[:, b:b + 128], tri[:])
                        nc.vector.tensor_mul(e4[:, b + QBS:b + QBS + 128],
                                             e4[:, b + QBS:b + QBS + 128],
                                             tri[:])
                return [(kt_i, off, N, b, e4) for (kt_i, off, N, b) in gs]

            def yt_group(hp, g, pyA, pyB, nkt):
                kt_i, off, N, b, e4 = g
                vA = vall[kt_i][:, (2 * hp) * 65:(2 * hp) * 65 + 65]
                vB = vall[kt_i][:, (2 * hp + 1) * 65:(2 * hp + 1) * 65 + 65]
                nc.tensor.matmul(pyA[:, off:QBS], vA, e4[:, b:b + N],
                                 start=(kt_i == 0), stop=(kt_i == nkt - 1))
                nc.tensor.matmul(pyB[:, off:QBS], vB,
                                 e4[:, b + QBS:b + QBS + N],
                                 start=(kt_i == 0), stop=(kt_i == nkt - 1))

            def attention_pair(hp, qt, kt, pump, finish_qb=None,
                               prev_tail=None, defer_tail=True):
                """Full attention for head-pair hp; pump(qb, kt_i) emits one
                unit of background work (or does nothing) and is called once
                per score group to hide the exp latency. The last q-block's
                normalization chain is deferred (returned as a closure) and
                emitted a few groups into the NEXT pair's q-block 0, where
                the Vector queue is clean -- keeping it off the pair
                boundary that stalls the next attn@V matmuls."""
                tail = None
                for qb in range(NQB):
                    q0 = qb * QBS
                    pyA = psyt.tile([65, QBS], F32, name="pyA", tag="ytA")
                    pyB = psyt.tile([65, QBS], F32, name="pyB", tag="ytB")
                    nfull = q0 // 128
                    nkt = nfull + QBS // 128
                    pend = None
                    for pi in range(nkt // 2):
                        gs = score_pair(qt, kt, pi, nfull, q0)
                        if qb == 1 and pi == 0 and prev_tail is not None:
                            # q-block 1's leading groups are non-diagonal (no
                            # tri-mask on Vector), so the previous pair's
                            # deferred normalization slots in here without
                            # delaying any attn@V dependency
                            prev_tail()
                            prev_tail = None
                        pump(qb, 2 * pi)
                        pump(qb, 2 * pi + 1)
                        if pend is not None:
                            yt_group(hp, pend[0], pyA, pyB, nkt)
                            yt_group(hp, pend[1], pyA, pyB, nkt)
                        pend = gs
                    yt_group(hp, pend[0], pyA, pyB, nkt)
                    yt_group(hp, pend[1], pyA, pyB, nkt)

                    # evict the psum accumulators via the Scalar engine (the
                    # Vector queue holds RoPE work that would delay the next
                    # q-block's first attn@V matmul); normalize from SBUF
                    yAsb = normp.tile([65, QBS], F32, name="yAsb",
                                      tag="yAsb", bufs=2)
                    yBsb = normp.tile([65, QBS], F32, name="yBsb",
                                      tag="yBsb", bufs=2)
                    nc.scalar.copy(yAsb[:], pyA[:])
                    nc.scalar.copy(yBsb[:], pyB[:])

                    def norm(qb=qb, yAsb=yAsb, yBsb=yBsb):
                        rA = normp.tile([1, 4 * QBS], F32, name="rA",
                                        tag="rA", bufs=1)
                        nc.vector.tensor_copy(rA[0:1, 0:QBS], yAsb[64:65, :])
                        nc.vector.tensor_copy(rA[0:1, QBS:2 * QBS],
                                              yBsb[64:65, :])
                        nc.vector.reciprocal_approx_fast(
                            rA[0:1, 2 * QBS:4 * QBS], rA[0:1, 0:2 * QBS])
                        bcst = normp.tile([64, 2 * QBS], F32, name="bcst",
                                          tag="bcst", bufs=1)
                        nc.gpsimd.partition_broadcast(
                            bcst[:], rA[0:1, 2 * QBS:4 * QBS], channels=64)
                        qbs = slice(qb * QBS, (qb + 1) * QBS)
                        nc.vector.tensor_mul(ytall[hp][0:64, qbs],
                                             yAsb[0:64, :],
                                             bcst[0:64, 0:QBS])
                        nc.vector.tensor_mul(ytall[hp][64:128, qbs],
                                             yBsb[0:64, :],
                                             bcst[0:64, QBS:2 * QBS])
                        if v_bias:
                            nc.vector.tensor_scalar_add(
                                ytall[hp][0:64, qbs], ytall[hp][0:64, qbs],
                                bv[0:64, hp:hp + 1])
                            nc.vector.tensor_scalar_add(
                                ytall[hp][64:128, qbs],
                                ytall[hp][64:128, qbs],
                                bv[64:128, hp:hp + 1])
                        if finish_qb is not None:
                            finish_qb(qb)

                    if qb == NQB - 1 and defer_tail:
                        tail = norm
                    else:
                        norm()
                if prev_tail is not None:
                    prev_tail()
                return tail

            with tc.tile_pool(name="xt", bufs=1) as xtp:
                # x^T resident, packed chunk-major; 4 bulk DMAs
                xt = xtp.tile([128, NTS * KC * TSW], BF16, name="xt",
                              tag="xt")

                def xmm(k, ts):
                    """[128, TSW] block of x^T: C-rows k, tokens slice ts."""
                    o = (ts * KC + k) * TSW
                    return xt[:, o:o + TSW]

                def xst(k, t):
                    """[128, 128] stationary block: C-rows k, token tile t."""
                    o = ((t // 4) * KC + k) * TSW + (t % 4) * 128
                    return xt[:, o:o + 128]

                # ========= phases 1+2: V projection and head pairs 0..6 ====
                # wv coexists with wqk/rope so pair-0's QKV projection can
                # run between the two V-projection halves, covering the
                # WAR-delayed DMA of wv half 1.
                with (
                    tc.tile_pool(name="wv", bufs=1) as wvp,
                    tc.tile_pool(name="wqk", bufs=1) as wqkp,
                    tc.tile_pool(name="rope", bufs=2) as ropep,
                ):
                    # the two wv halves share one slot (bufs=1): half 1's DMA
                    # carries a WAR wait on half 0's readers, so it must be
                    # issued after every no-dependency DMA or it blocks the
                    # Sync queue behind it
                    wvh = []
                    for h in range(NVS):
                        w = wvp.tile([128, KC * VN], BF16, name=f"wvh{h}",
                                     tag="wvh")
                        wvh.append(w)
                    # fine-grained first-need DMAs: the first matmul chain is
                    # paced by the k-blocks of wv half 0 + xt chunk 0
                    # first-need payload in 8+8 fine pieces: wv triggers on
                    # Sync, xt triggers on Scalar (parallel trigger issue,
                    # more DMA queues in flight)
                    for j in range(8):
                        o = j * 2 * VN
                        nc.sync.dma_start(wvh[0][:, o:o + 2 * VN],
                                          wv_d.ap()[0][:, o:o + 2 * VN])
                        o = j * 2 * TSW
                        nc.scalar.dma_start(xt[:, o:o + 2 * TSW],
                                            xt_d.ap()[:, o:o + 2 * TSW])
                    for c in range(1, NTS):
                        o = c * KC * TSW
                        nc.sync.dma_start(xt[:, o:o + KC * TSW],
                                          xt_d.ap()[:, o:o + KC * TSW])

                    def vproj(ns):
                        for t in range(TT):
                            v3 = vall[t][:].rearrange("p (h c) -> p h c", c=65)
                            if ns == 0:
                                nc.vector.memset(v3[:, :, 64:65], 1.0)
                            pv = psmm.tile([128, VN], F32, name="pv", tag="mm")
                            for k in range(KC):
                                nc.tensor.matmul(
                                    pv[:], xst(k, t),
                                    wvh[ns][:, k * VN:(k + 1) * VN],
                                    start=(k == 0), stop=(k == KC - 1),
                                )
                            nh = VN // 64
                            src = pv[:].rearrange("p (h c) -> p h c", c=64)
                            dst = v3[:, ns * nh:(ns + 1) * nh, 0:64]
                            nc.vector.tensor_copy(dst, src)

                    def new_qkv(hp):
                        """Allocate tiles + DMA for head-pair hp; return
                        (qt, kt, step-generator emitting QKV matmuls + rope)."""
                        wq = wqkp.tile([128, KC * 128], BF16, name="wq",
                                       tag="wq")
                        wk = wqkp.tile([128, KC * 128], BF16, name="wk",
                                       tag="wk")
                        nc.sync.dma_start(wq[:], wq_d.ap()[hp])
                        nc.sync.dma_start(wk[:], wk_d.ap()[hp])
                        qt = qktp.tile([128, T], BF16, name="qt", tag="qt")
                        kt = qktp.tile([128, T], BF16, name="kt", tag="kt")

                        def steps():
                            # ts-major so early token slices of BOTH q and k
                            # land first
                            for ts in range(NTS):
                                for (wsb, dst, bcol) in ((wq, qt, hp),
                                                         (wk, kt, HP + hp)):
                                    sl = slice(ts * TSW, (ts + 1) * TSW)
                                    pq = psmm.tile([128, TSW], F32, name="pq",
                                                   tag="mm")

                                    def mm(k):
                                        nc.tensor.matmul(
                                            pq[:],
                                            wsb[:, k * 128:(k + 1) * 128],
                                            xmm(k, ts),
                                            start=(k == 0),
                                            stop=(k == KC - 1),
                                        )

                                    head = list(range(KC - 4))
                                    for k0 in range(0, len(head), 4):
                                        for k in head[k0:k0 + 4]:
                                            mm(k)
                                        yield
                                    for k in (KC - 4, KC - 3):
                                        mm(k)
                                    yield
                                    for k in (KC - 2, KC - 1):
                                        mm(k)
                                    raw = ropep.tile([128, TSW], BF16,
                                                     name="raw", tag="raw")
                                    nc.vector.tensor_copy(raw[:], pq[:])
                                    if qk_bias:
                                        nc.vector.tensor_scalar_add(
                                            raw[:], raw[:],
                                            bqk[:, bcol:bcol + 1])
                                    t1 = ropep.tile([128, TSW], BF16,
                                                    name="t1", tag="t1")
                                    nc.vector.tensor_mul(t1[:], raw[:],
                                                         cosf[:, sl])
                                    # sinf rows are host-swapped (row r holds
                                    # the sin for destination row r^32) so both
                                    # inputs read at the same base partition.
                                    t2 = ropep.tile([128, TSW], BF16,
                                                    name="t2", tag="t2")
                                    for blk in range(4):
                                        sb_ = blk ^ 1
                                        nc.vector.tensor_mul(
                                            t2[blk * 32:(blk + 1) * 32, :],
                                            raw[sb_ * 32:(sb_ + 1) * 32, :],
                                            sinf[sb_ * 32:(sb_ + 1) * 32, sl],
                                        )
                                    nc.vector.tensor_add(dst[:, sl], t1[:],
                                                         t2[:])
                                    yield

                        return qt, kt, steps()

                    vproj(0)
                    cur = new_qkv(0)      # wq/wk DMAs flow before wvh1's
                    for h in range(1, NVS):
                        nc.sync.dma_start(wvh[h][:], wv_d.ap()[h])
                    for _ in cur[2]:      # pair-0 QKV covers wvh1's DMA
                        pass
                    vproj(1)

                    ptail = None
                    for hp in range(HP - 1):
                        qt, kt = cur[0], cur[1]
                        nxt = new_qkv(hp + 1)
                        bg = nxt[2]

                        def pump(qb, kt_i, bg=bg):
                            # the first background steps can wait on the wq/wk
                            # DMA; pumping them too early would block the
                            # in-order PE queue behind that wait. Double-pump
                            # late q-blocks so the generator is fully drained
                            # before the pair boundary (no Vector-queue dump).
                            if qb == 0 and kt_i < 3:
                                return
                            next(bg, None)
                            if qb >= 2:
                                next(bg, None)

                        ptail = attention_pair(hp, qt, kt, pump,
                                               prev_tail=ptail)
                        for _ in bg:
                            pass
                        cur = nxt

            # ========= phase 3: pair 7 attention + out projection =========
            # xt/wqk/rope pools are closed; prefetch the out-proj weights and
            # pump out-proj matmul groups as pair 7's background work.
            with (
                tc.tile_pool(name="wo", bufs=1) as wop,
                tc.tile_pool(name="ost", bufs=4) as ostp,
            ):
                wo = wop.tile([128, DC * C], BF16, name="wo", tag="wo")
                nc.sync.dma_start(wo[:], wo_d.ap())

                oq = []

                def emit_oproj():
                    t_, cs = oq.pop(0)
                    po = psmm.tile([128, CS], F32, name="po", tag="mm")
                    for d_ in range(DC):
                        nc.tensor.matmul(
                            po[:],
                            ytall[d_][:, t_ * 128:(t_ + 1) * 128],
                            wo[:, d_ * C + cs * CS:d_ * C + (cs + 1) * CS],
                            start=(d_ == 0), stop=(d_ == DC - 1),
                        )
                    st = ostp.tile([128, CS], BF16, name="st", tag="ost")
                    nc.scalar.copy(st[:], po[:])
                    nc.sync.dma_start(
                        out_d.ap()[t_ * 128:(t_ + 1) * 128,
                                   cs * CS:(cs + 1) * CS],
                        st[:])

                def pump7(qb, kt_i):
                    # wo's 4MB DMA is still in flight during q-block 1's
                    # first groups; a waiting out-proj matmul would block
                    # the in-order PE queue
                    if qb == 1 and kt_i < 5:
                        return
                    if oq:
                        emit_oproj()

                def finish_qb(qb):
                    # t-tiles covered by this q-block are now complete for
                    # every head pair -> eligible for the out projection
                    for t_ in range(qb * (QBS // 128),
                                    (qb + 1) * (QBS // 128)):
                        for cs in range(NCS):
                            oq.append((t_, cs))

                hp = HP - 1
                attention_pair(hp, cur[0], cur[1], pump7, finish_qb,
                               prev_tail=ptail, defer_tail=False)

                while oq:
                    emit_oproj()

    nc.compile()
    return nc


# ---------------------------------------------------------------------------
# host-side sharding
# ---------------------------------------------------------------------------

def _planar_perm():
    """Within-head column permutation: even dims -> 0..31, odd -> 32..63."""
    p = np.empty(HEAD_DIM, dtype=np.int64)
    p[:32] = 2 * np.arange(32)
    p[32:] = 2 * np.arange(32) + 1
    return p


def _rope_tables(T):
    theta = 1.0 / (10000.0 ** (np.arange(0, HEAD_DIM, 2, dtype=np.float64)
                               / HEAD_DIM))  # [32]
    idx = np.outer(np.arange(T, dtype=np.float64), theta)  # [T, 32]
    cos = np.cos(idx).astype(np.float32)
    sin = np.sin(idx).astype(np.float32)
    cosf = np.empty((128, T), dtype=np.float32)
    sinf = np.empty((128, T), dtype=np.float32)
    for r in range(128):
        i = r % 32
        lo = ((r // 32) % 2 == 0)
        cosf[r] = cos[:, i]
        sinf[r] = (-sin[:, i]) if lo else sin[:, i]
    # device reads the sin table at the *source* rows of the pair swap
    # (row r holds the value destined for row r^32), so swap 32-row blocks
    sinf = sinf.reshape(4, 32, T)[[1, 0, 3, 2]].reshape(128, T)
    return cosf, sinf


def make_in_maps(x, W_qkv, b_qkv, W_proj, T, C, HLOC, qk_bias, v_bias):
    B = x.shape[0]
    D = HEAD_DIM
    HP = HLOC // 2
    CL = HLOC * D
    KC = C // 128
    DC = CL // 128
    NTS = T // 512
    VN = min(512, CL)
    NVS = CL // VN
    NGRP = (C // D) // HLOC  # head groups

    Wq = np.asarray(W_qkv[:, 0:C], dtype=np.float32)
    Wk = np.asarray(W_qkv[:, C:2 * C], dtype=np.float32)
    Wv = np.asarray(W_qkv[:, 2 * C:3 * C], dtype=np.float32)
    bq = np.asarray(b_qkv[0:C], dtype=np.float32)
    bk = np.asarray(b_qkv[C:2 * C], dtype=np.float32)
    bv_ = np.asarray(b_qkv[2 * C:3 * C], dtype=np.float32)

    perm = _planar_perm()
    cosf, sinf = _rope_tables(T)
    tri = (np.arange(128)[:, None] <= np.arange(128)[None, :])

    def to_bf(a):
        return np.ascontiguousarray(a.astype(NPBF16))

    grp = {}
    for g in range(NGRP):
        cols_qk = np.concatenate(
            [(g * HLOC + h) * D + perm for h in range(HLOC)])
        cols_v = np.concatenate(
            [(g * HLOC + h) * D + np.arange(D) for h in range(HLOC)])
        wq_g = Wq[:, cols_qk]   # [C, CL]
        wk_g = Wk[:, cols_qk]
        wv_g = Wv[:, cols_v]
        wo_g = np.asarray(W_proj[g * CL:(g + 1) * CL, :], dtype=np.float32)

        ent = {
            "wq": to_bf(wq_g.reshape(KC, 128, HP, 128)
                        .transpose(2, 1, 0, 3).reshape(HP, 128, KC * 128)),
            "wk": to_bf(wk_g.reshape(KC, 128, HP, 128)
                        .transpose(2, 1, 0, 3).reshape(HP, 128, KC * 128)),
            # [NVS, 128, KC*VN]: half h holds column-slice h of every k-tile
            "wv": to_bf(wv_g.reshape(KC, 128, NVS, VN)
                        .transpose(2, 1, 0, 3).reshape(NVS, 128, KC * VN)),
            # [128, DC*C]: d-major packing of the DC row-tiles
            "wo": to_bf(wo_g.reshape(DC, 128, C)
                        .transpose(1, 0, 2).reshape(128, DC * C)),
            "cosf": to_bf(cosf),
            "sinf": to_bf(sinf),
            "tri": to_bf(tri.astype(np.float32)),
        }
        if qk_bias:
            bqk_t = np.empty((128, 2 * HP), dtype=np.float32)
            bq_g = bq[cols_qk]
            bk_g = bk[cols_qk]
            for hp in range(HP):
                bqk_t[:, hp] = bq_g[hp * 128:(hp + 1) * 128]
                bqk_t[:, HP + hp] = bk_g[hp * 128:(hp + 1) * 128]
            ent["bqk"] = bqk_t
        if v_bias:
            bv_t = np.empty((128, HP), dtype=np.float32)
            bv_g = bv_[cols_v]
            for hp in range(HP):
                bv_t[:, hp] = bv_g[hp * 128:(hp + 1) * 128]
            ent["bv"] = bv_t
        grp[g] = ent

    in_maps = []
    for core in range(B * NGRP):
        b, g = core // NGRP, core % NGRP
        m = dict(grp[g])
        # x^T packed chunk-major: [128, (ts, k, 512)]
        xtT = np.asarray(x[b], dtype=np.float32).T  # [C, T]
        m["xt"] = to_bf(xtT.reshape(KC, 128, NTS, 512)
                        .transpose(1, 2, 0, 3).reshape(128, NTS * KC * 512))
        in_maps.append(m)
    return in_maps


_CACHE = {}


def _get_graph(T, C, HLOC, qk_bias, v_bias):
    key = (T, C, HLOC, qk_bias, v_bias)
    if key not in _CACHE:
        _CACHE[key] = build_graph(T, C, HLOC, qk_bias, v_bias)
    return _CACHE[key]


def _ensure_ntff_hook():
    """Register the axon NTFF profile hook if the image's antenv lacks it."""
    import sys
    import types
    import antenv
    try:
        from antenv import axon_hooks  # noqa: F401
    except ImportError:
        mod = types.ModuleType("antenv.axon_hooks")
        mod._hook = None

        def set_axon_ntff_profile_hook(h, _m=mod):
            _m._hook = h

        def get_axon_ntff_profile_hook(_m=mod):
            return _m._hook

        mod.set_axon_ntff_profile_hook = set_axon_ntff_profile_hook
        mod.get_axon_ntff_profile_hook = get_axon_ntff_profile_hook
        sys.modules["antenv.axon_hooks"] = mod
        antenv.axon_hooks = mod
    from antenv.axon_hooks import (get_axon_ntff_profile_hook,
                                   set_axon_ntff_profile_hook)
    if get_axon_ntff_profile_hook() is None:
        from trn_agent_boot.trn_boot import _ntff_profile_via_ctypes
        set_axon_ntff_profile_hook(
            _ntff_profile_via_ctypes("/opt/axon/libaxon_pjrt.so"))


def run(inputs, trace=False):
    from concourse.bass_utils import run_bass_kernel_spmd
    if trace:
        try:
            _ensure_ntff_hook()
        except Exception as e:
            print(f"ntff hook setup failed: {e}")
    x = np.asarray(inputs["x"])
    W_qkv = np.asarray(inputs["W_qkv"])
    b_qkv = np.asarray(inputs["b_qkv"])
    W_proj = np.asarray(inputs["W_proj"])
    b_proj = np.asarray(inputs["b_proj"])
    B, T, C = x.shape
    HLOC = HLOC_FULL
    NGRP = (C // HEAD_DIM) // HLOC

    qk_bias = bool(np.any(b_qkv[0:2 * C]))
    v_bias = bool(np.any(b_qkv[2 * C:]))
    nc = _get_graph(T, C, HLOC, qk_bias, v_bias)
    in_maps = make_in_maps(x, W_qkv, b_qkv, W_proj, T, C, HLOC,
                           qk_bias, v_bias)
    res = run_bass_kernel_spmd(nc, in_maps, core_ids=list(range(len(in_maps))),
                               trace=trace)
    out = np.empty((B, T, C), dtype=np.float32)
    for b in range(B):
        acc = None
        for g in range(NGRP):
            part = np.asarray(res.results[b * NGRP + g]["out"],
                              dtype=np.float32)
            acc = part if acc is None else acc + part
        out[b] = acc + b_proj[None, :].astype(np.float32)
    return out, res


def kernel(**inputs):
    out, _ = run(inputs, trace=False)
    return out


# revision 45
# speedup vs baseline: 1.4028x; 1.4028x over previous
"""Trainium2 Bass kernel: causal multi-head attention block (QKV proj + RoPE +
causal softmax attention + out proj), distributed over 8 NeuronCores.

Sharding: core = (batch b in 0..3, head-group g in 0..1). Each core computes
the full attention pipeline for its batch and its 16 heads, producing a
partial [T, C] output (its heads' contribution through the out projection).
Host sums the two partials per batch and adds b_proj. No collectives.

Per-core layouts (host-prepared, mostly bf16):
  xt   [128, NTS*KC*512]  x[b].T packed chunk-major: block (ts, k) holds
                          C-rows k*128:(k+1)*128 x tokens ts*512:(ts+1)*512.
                          One DMA per token-chunk -> compute starts after the
                          first 2MB instead of the full 8MB.
  wq   [HP, 128, KC*128]  W_q columns for group g, RoPE-planar-permuted, tiled
  wk   [HP, 128, KC*128]  same for K
  wv   [NVS, 128, KC*VN]  W_v columns for group g, packed per ns-half
  wo   [128, DC*C]        W_proj rows for group g, packed d-major
  cosf/sinf [128, T]      RoPE tables in head-pair row layout (sign folded)
  tri  [128, 128]         causal mask for diagonal blocks (kk <= qq)
  out  [T, C] bf16        partial output

Schedule: phase 1 (V proj, ns-outer so only wv-half0 + xt-chunk0 gate the
first matmul), pairs 0..6 attention with next-pair QKV pumped as background,
then pair 7 attention in a narrower scope (xt/wqk/rope pools closed) with the
out-projection weights prefetched and out-proj matmul groups pumped as its
background work, then the rest of the out projection. PSUM evictions of the
attention accumulators go through the Scalar engine so they are not queued
behind RoPE work on Vector (that stalled the PE at q-block boundaries).
"""

import numpy as np
import ml_dtypes

import concourse.bass as bass
import concourse.bacc as bacc
import concourse.mybir as mybir
import concourse.tile as tile

BF16 = mybir.dt.bfloat16
F32 = mybir.dt.float32
AF = mybir.ActivationFunctionType
NPBF16 = ml_dtypes.bfloat16

N_EMBD = 2048
N_HEAD = 32
HEAD_DIM = 64
B_FULL = 4
T_FULL = 2048
N_CORES = 8
HLOC_FULL = 16  # heads per core


def build_graph(T=2048, C=2048, HLOC=16, qk_bias=False, v_bias=False):
    D = HEAD_DIM
    HP = HLOC // 2          # head pairs per core
    CL = HLOC * D           # local head dims
    KC = C // 128           # contraction chunks for projections
    TT = T // 128           # token tiles
    QBS = min(512, T)       # q-block size
    NQB = T // QBS
    TSW = min(512, T)       # token slice width for qkv psum
    NTS = T // TSW
    VN = min(512, CL)       # v matmul free width
    NVS = CL // VN
    DC = CL // 128          # out-proj contraction chunks
    CS = min(512, C)        # out-proj col slice
    NCS = C // CS
    SCALE = 1.0 / float(np.sqrt(D))

    nc = bacc.Bacc(None, target_bir_lowering=False, debug=False)

    xt_d = nc.declare_dram_parameter("xt", [128, NTS * KC * TSW], BF16, False)
    wq_d = nc.declare_dram_parameter("wq", [HP, 128, KC * 128], BF16, False)
    wk_d = nc.declare_dram_parameter("wk", [HP, 128, KC * 128], BF16, False)
    wv_d = nc.declare_dram_parameter("wv", [NVS, 128, KC * VN], BF16, False)
    wo_d = nc.declare_dram_parameter("wo", [128, DC * C], BF16, False)
    cos_d = nc.declare_dram_parameter("cosf", [128, T], BF16, False)
    sin_d = nc.declare_dram_parameter("sinf", [128, T], BF16, False)
    tri_d = nc.declare_dram_parameter("tri", [128, 128], BF16, False)
    if qk_bias:
        bqk_d = nc.declare_dram_parameter("bqk", [128, 2 * HP], F32, False)
    if v_bias:
        bv_d = nc.declare_dram_parameter("bv", [128, HP], F32, False)
    out_d = nc.declare_dram_parameter("out", [T, C], BF16, True)

    with tile.TileContext(nc) as tc:
        with (
            tc.tile_pool(name="const", bufs=1) as constp,
            tc.tile_pool(name="vall", bufs=1) as vallp,
            tc.tile_pool(name="yt", bufs=1) as ytp,
            tc.tile_pool(name="qkt", bufs=2) as qktp,
            tc.tile_pool(name="esc", bufs=2) as ep,
            tc.tile_pool(name="norm", bufs=1) as normp,
            tc.tile_pool(name="psmm", bufs=2, space="PSUM") as psmm,
            tc.tile_pool(name="pssc", bufs=2, space="PSUM") as pssc,
            tc.tile_pool(name="psyt", bufs=1, space="PSUM") as psyt,
        ):
            # ---- constants ----
            cosf = constp.tile([128, T], BF16, name="cosf", tag="cosf")
            sinf = constp.tile([128, T], BF16, name="sinf", tag="sinf")
            tri = constp.tile([128, 128], BF16, name="tri", tag="tri")
            nc.sync.dma_start(cosf[:], cos_d.ap())
            nc.sync.dma_start(sinf[:], sin_d.ap())
            nc.sync.dma_start(tri[:], tri_d.ap())
            if qk_bias:
                bqk = constp.tile([128, 2 * HP], F32, name="bqk", tag="bqk")
                nc.sync.dma_start(bqk[:], bqk_d.ap())
            if v_bias:
                bv = constp.tile([128, HP], F32, name="bv", tag="bv")
                nc.sync.dma_start(bv[:], bv_d.ap())

            # ---- v_all tiles (65-packed: 64 dims + ones column per head) ----
            vall = []
            for t in range(TT):
                vt = vallp.tile([128, HLOC * 65], BF16, name=f"vall{t}",
                                tag=f"vall{t}")
                vall.append(vt)

            # ---- yT accumulator tiles ----
            ytall = []
            for d_ in range(DC):
                yt_ = ytp.tile([128, T], BF16, name=f"yt{d_}", tag=f"yt{d_}")
                ytall.append(yt_)

            # =============== shared attention helpers ===============
            def geom(kt_i, nfull):
                if kt_i < nfull:
                    return 0, QBS
                i = kt_i - nfull
                return 128 * i, QBS - 128 * i

            def score_pair(qt, kt, pi, nfull, q0):
                """Score matmuls + merged exp + tri-masks for groups
                (2*pi, 2*pi+1) sharing one 4-bank psum quad and one e tile.
                Merging the two groups' exps into one ACT call saves the
                per-call access latency + semaphore wait on the engine that
                paces the attn@V matmuls."""
                sc4 = pssc.tile([128, 4 * QBS], F32, name="sc4", tag="sc",
                                bufs=1)
                e4 = ep.tile([128, 4 * QBS], BF16, name="e4", tag="e",
                             bufs=2)
                gs = []
                for half, kt_i in ((0, 2 * pi), (1, 2 * pi + 1)):
                    off, N = geom(kt_i, nfull)
                    ksl = slice(kt_i * 128, (kt_i + 1) * 128)
                    qsl = slice(q0 + off, q0 + QBS)
                    b = half * 2 * QBS
                    nc.tensor.matmul(sc4[:, b:b + N], kt[0:64, ksl],
                                     qt[0:64, qsl], start=True, stop=True)
                    nc.tensor.matmul(sc4[:, b + QBS:b + QBS + N],
                                     kt[64:128, ksl], qt[64:128, qsl],
                                     start=True, stop=True)
                    gs.append((kt_i, off, N, b))
                Na, Nb = gs[0][2], gs[1][2]
                if Na + Nb > 512:
                    span = 3 * QBS + Nb
                    nc.scalar.activation(e4[:, 0:span], sc4[:, 0:span],
                                         AF.Exp, scale=SCALE)
                else:
                    for (kt_i, off, N, b) in gs:
                        nc.scalar.activation(e4[:, b:b + N], sc4[:, b:b + N],
                                             AF.Exp, scale=SCALE)
                        nc.scalar.activation(e4[:, b + QBS:b + QBS + N],
                                             sc4[:, b + QBS:b + QBS + N],
                                             AF.Exp, scale=SCALE)
                for (kt_i, off, N, b) in gs:
                    if kt_i >= nfull:
                        nc.vector.tensor_mul(e4[:, b:b + 128],
                                             e4[:, b:b + 128], tri[:])
                        nc.vector.tensor_mul(e4[:, b + QBS:b + QBS + 128],
                                             e4[:, b + QBS:b + QBS + 128],
                                             tri[:])
                return [(kt_i, off, N, b, e4) for (kt_i, off, N, b) in gs]

            def yt_group(hp, g, pyA, pyB, nkt):
                kt_i, off, N, b, e4 = g
                vA = vall[kt_i][:, (2 * hp) * 65:(2 * hp) * 65 + 65]
                vB = vall[kt_i][:, (2 * hp + 1) * 65:(2 * hp + 1) * 65 + 65]
                nc.tensor.matmul(pyA[:, off:QBS], vA, e4[:, b:b + N],
                                 start=(kt_i == 0), stop=(kt_i == nkt - 1))
                nc.tensor.matmul(pyB[:, off:QBS], vB,
                                 e4[:, b + QBS:b + QBS + N],
                                 start=(kt_i == 0), stop=(kt_i == nkt - 1))

            def attention_pair(hp, qt, kt, pump, finish_qb=None,
                               prev_tail=None, defer_tail=True):
                """Full attention for head-pair hp; pump(qb, kt_i) emits one
                unit of background work (or does nothing) and is called once
                per score group to hide the exp latency. The last q-block's
                normalization chain is deferred (returned as a closure) and
                emitted a few groups into the NEXT pair's q-block 0, where
                the Vector queue is clean -- keeping it off the pair
                boundary that stalls the next attn@V matmuls."""
                tail = None
                for qb in range(NQB):
                    q0 = qb * QBS
                    pyA = psyt.tile([65, QBS], F32, name="pyA", tag="ytA")
                    pyB = psyt.tile([65, QBS], F32, name="pyB", tag="ytB")
                    nfull = q0 // 128
                    nkt = nfull + QBS // 128
                    pend = None
                    for pi in range(nkt // 2):
                        gs = score_pair(qt, kt, pi, nfull, q0)
                        if qb == 1 and pi == 0 and prev_tail is not None:
                            # q-block 1's leading groups are non-diagonal (no
                            # tri-mask on Vector), so the previous pair's
                            # deferred normalization slots in here without
                            # delaying any attn@V dependency
                            prev_tail()
                            prev_tail = None
                        pump(qb, 2 * pi)
                        pump(qb, 2 * pi + 1)
                        if pend is not None:
                            yt_group(hp, pend[0], pyA, pyB, nkt)
                            yt_group(hp, pend[1], pyA, pyB, nkt)
                        pend = gs
                    yt_group(hp, pend[0], pyA, pyB, nkt)
                    yt_group(hp, pend[1], pyA, pyB, nkt)

                    # evict the psum accumulators via the Scalar engine (the
                    # Vector queue holds RoPE work that would delay the next
                    # q-block's first attn@V matmul); normalize from SBUF
                    yAsb = normp.tile([65, QBS], F32, name="yAsb",
                                      tag="yAsb", bufs=2)
                    yBsb = normp.tile([65, QBS], F32, name="yBsb",
                                      tag="yBsb", bufs=2)
                    nc.scalar.copy(yAsb[:], pyA[:])
                    nc.scalar.copy(yBsb[:], pyB[:])
                    # stage the denominator rows (tensor_copy handles the
                    # base-partition shift; the custom-DVE reciprocal does
                    # not), then take the reciprocal in place
                    rA = normp.tile([1, 2 * QBS], F32, name="rA",
                                    tag="rA", bufs=2)
                    nc.vector.tensor_copy(rA[0:1, 0:QBS], yAsb[64:65, :])
                    nc.vector.tensor_copy(rA[0:1, QBS:2 * QBS],
                                          yBsb[64:65, :])
                    nc.vector.reciprocal_approx_fast(rA[0:1, :], rA[0:1, :])

                    def norm(qb=qb, yAsb=yAsb, yBsb=yBsb, rA=rA):
                        bcst = normp.tile([64, 2 * QBS], F32, name="bcst",
                                          tag="bcst", bufs=1)
                        nc.gpsimd.partition_broadcast(
                            bcst[:], rA[0:1, 0:2 * QBS], channels=64)
                        qbs = slice(qb * QBS, (qb + 1) * QBS)
                        nc.vector.tensor_mul(ytall[hp][0:64, qbs],
                                             yAsb[0:64, :],
                                             bcst[0:64, 0:QBS])
                        nc.vector.tensor_mul(ytall[hp][64:128, qbs],
                                             yBsb[0:64, :],
                                             bcst[0:64, QBS:2 * QBS])
                        if v_bias:
                            nc.vector.tensor_scalar_add(
                                ytall[hp][0:64, qbs], ytall[hp][0:64, qbs],
                                bv[0:64, hp:hp + 1])
                            nc.vector.tensor_scalar_add(
                                ytall[hp][64:128, qbs],
                                ytall[hp][64:128, qbs],
                                bv[64:128, hp:hp + 1])
                        if finish_qb is not None:
                            finish_qb(qb)

                    if qb == NQB - 1 and defer_tail:
                        tail = norm
                    else:
                        norm()
                if prev_tail is not None:
                    prev_tail()
                return tail

            with tc.tile_pool(name="xt", bufs=1) as xtp:
                # x^T resident, packed chunk-major; 4 bulk DMAs
                xt = xtp.tile([128, NTS * KC * TSW], BF16, name="xt",
                              tag="xt")

                def xmm(k, ts):
                    """[128, TSW] block of x^T: C-rows k, tokens slice ts."""
                    o = (ts * KC + k) * TSW
                    return xt[:, o:o + TSW]

                def xst(k, t):
                    """[128, 128] stationary block: C-rows k, token tile t."""
                    o = ((t // 4) * KC + k) * TSW + (t % 4) * 128
                    return xt[:, o:o + 128]

                # ========= phases 1+2: V projection and head pairs 0..6 ====
                # wv coexists with wqk/rope so pair-0's QKV projection can
                # run between the two V-projection halves, covering the
                # WAR-delayed DMA of wv half 1.
                with (
                    tc.tile_pool(name="wv", bufs=1) as wvp,
                    tc.tile_pool(name="wqk", bufs=1) as wqkp,
                    tc.tile_pool(name="rope", bufs=1) as ropep,
                ):
                    # the two wv halves share one slot (bufs=1): half 1's DMA
                    # carries a WAR wait on half 0's readers, so it must be
                    # issued after every no-dependency DMA or it blocks the
                    # Sync queue behind it
                    wvh = []
                    for h in range(NVS):
                        w = wvp.tile([128, KC * VN], BF16, name=f"wvh{h}",
                                     tag="wvh")
                        wvh.append(w)
                    # fine-grained first-need DMAs: the first matmul chain is
                    # paced by the k-blocks of wv half 0 + xt chunk 0
                    # first-need payload in 8+8 fine pieces: wv triggers on
                    # Sync, xt triggers on Scalar (parallel trigger issue,
                    # more DMA queues in flight)
                    for j in range(8):
                        o = j * 2 * VN
                        nc.sync.dma_start(wvh[0][:, o:o + 2 * VN],
                                          wv_d.ap()[0][:, o:o + 2 * VN])
                        o = j * 2 * TSW
                        nc.scalar.dma_start(xt[:, o:o + 2 * TSW],
                                            xt_d.ap()[:, o:o + 2 * TSW])
                    for c in range(1, NTS):
                        o = c * KC * TSW
                        nc.sync.dma_start(xt[:, o:o + KC * TSW],
                                          xt_d.ap()[:, o:o + KC * TSW])

                    def vproj(ns):
                        for t in range(TT):
                            v3 = vall[t][:].rearrange("p (h c) -> p h c", c=65)
                            if ns == 0:
                                nc.vector.memset(v3[:, :, 64:65], 1.0)
                            pv = psmm.tile([128, VN], F32, name="pv", tag="mm")
                            for k in range(KC):
                                nc.tensor.matmul(
                                    pv[:], xst(k, t),
                                    wvh[ns][:, k * VN:(k + 1) * VN],
                                    start=(k == 0), stop=(k == KC - 1),
                                )
                            nh = VN // 64
                            src = pv[:].rearrange("p (h c) -> p h c", c=64)
                            dst = v3[:, ns * nh:(ns + 1) * nh, 0:64]
                            nc.vector.tensor_copy(dst, src)

                    def new_qkv(hp):
                        """Allocate tiles + DMA for head-pair hp; return
                        (qt, kt, step-generator emitting QKV matmuls + rope)."""
                        wq = wqkp.tile([128, KC * 128], BF16, name="wq",
                                       tag="wq")
                        wk = wqkp.tile([128, KC * 128], BF16, name="wk",
                                       tag="wk")
                        nc.sync.dma_start(wq[:], wq_d.ap()[hp])
                        nc.sync.dma_start(wk[:], wk_d.ap()[hp])
                        qt = qktp.tile([128, T], BF16, name="qt", tag="qt")
                        kt = qktp.tile([128, T], BF16, name="kt", tag="kt")

                        def steps():
                            # ts-major so early token slices of BOTH q and k
                            # land first
                            for ts in range(NTS):
                                for (wsb, dst, bcol) in ((wq, qt, hp),
                                                         (wk, kt, HP + hp)):
                                    sl = slice(ts * TSW, (ts + 1) * TSW)
                                    pq = psmm.tile([128, TSW], F32, name="pq",
                                                   tag="mm")

                                    def mm(k):
                                        nc.tensor.matmul(
                                            pq[:],
                                            wsb[:, k * 128:(k + 1) * 128],
                                            xmm(k, ts),
                                            start=(k == 0),
                                            stop=(k == KC - 1),
                                        )

                                    head = list(range(KC - 4))
                                    for k0 in range(0, len(head), 4):
                                        for k in head[k0:k0 + 4]:
                                            mm(k)
                                        yield
                                    for k in (KC - 4, KC - 3):
                                        mm(k)
                                    yield
                                    for k in (KC - 2, KC - 1):
                                        mm(k)
                                    raw = ropep.tile([128, TSW], BF16,
                                                     name="raw", tag="raw")
                                    nc.vector.tensor_copy(raw[:], pq[:])
                                    if qk_bias:
                                        nc.vector.tensor_scalar_add(
                                            raw[:], raw[:],
                                            bqk[:, bcol:bcol + 1])
                                    t1 = ropep.tile([128, TSW], BF16,
                                                    name="t1", tag="t1")
                                    nc.vector.tensor_mul(t1[:], raw[:],
                                                         cosf[:, sl])
                                    # sinf rows are host-swapped (row r holds
                                    # the sin for destination row r^32) so both
                                    # inputs read at the same base partition.
                                    t2 = ropep.tile([128, TSW], BF16,
                                                    name="t2", tag="t2")
                                    for blk in range(4):
                                        sb_ = blk ^ 1
                                        nc.vector.tensor_mul(
                                            t2[blk * 32:(blk + 1) * 32, :],
                                            raw[sb_ * 32:(sb_ + 1) * 32, :],
                                            sinf[sb_ * 32:(sb_ + 1) * 32, sl],
                                        )
                                    nc.vector.tensor_add(dst[:, sl], t1[:],
                                                         t2[:])
                                    yield

                        return qt, kt, steps()

                    vproj(0)
                    cur = new_qkv(0)      # wq/wk DMAs flow before wvh1's
                    for h in range(1, NVS):
                        nc.sync.dma_start(wvh[h][:], wv_d.ap()[h])
                    for _ in cur[2]:      # pair-0 QKV covers wvh1's DMA
                        pass
                    vproj(1)

                    ptail = None
                    for hp in range(HP - 1):
                        qt, kt = cur[0], cur[1]
                        nxt = new_qkv(hp + 1)
                        bg = nxt[2]

                        def pump(qb, kt_i, bg=bg):
                            # the first background steps can wait on the wq/wk
                            # DMA; pumping them too early would block the
                            # in-order PE queue behind that wait. Double-pump
                            # late q-blocks so the generator is fully drained
                            # before the pair boundary (no Vector-queue dump).
                            if qb == 0 and kt_i < 3:
                                return
                            next(bg, None)
                            if qb >= 2:
                                next(bg, None)

                        ptail = attention_pair(hp, qt, kt, pump,
                                               prev_tail=ptail)
                        for _ in bg:
                            pass
                        cur = nxt

            # ========= phase 3: pair 7 attention + out projection =========
            # xt/wqk/rope pools are closed; prefetch the out-proj weights and
            # pump out-proj matmul groups as pair 7's background work.
            with (
                tc.tile_pool(name="wo", bufs=1) as wop,
                tc.tile_pool(name="ost", bufs=4) as ostp,
            ):
                wo = wop.tile([128, DC * C], BF16, name="wo", tag="wo")
                nc.sync.dma_start(wo[:], wo_d.ap())

                oq = []

                def emit_oproj():
                    t_, cs = oq.pop(0)
                    po = psmm.tile([128, CS], F32, name="po", tag="mm")
                    for d_ in range(DC):
                        nc.tensor.matmul(
                            po[:],
                            ytall[d_][:, t_ * 128:(t_ + 1) * 128],
                            wo[:, d_ * C + cs * CS:d_ * C + (cs + 1) * CS],
                            start=(d_ == 0), stop=(d_ == DC - 1),
                        )
                    st = ostp.tile([128, CS], BF16, name="st", tag="ost")
                    nc.scalar.copy(st[:], po[:])
                    nc.sync.dma_start(
                        out_d.ap()[t_ * 128:(t_ + 1) * 128,
                                   cs * CS:(cs + 1) * CS],
                        st[:])

                def pump7(qb, kt_i):
                    # wo's 4MB DMA is still in flight during q-block 1's
                    # first groups; a waiting out-proj matmul would block
                    # the in-order PE queue
                    if qb == 1 and kt_i < 5:
                        return
                    if oq:
                        emit_oproj()

                def finish_qb(qb):
                    # t-tiles covered by this q-block are now complete for
                    # every head pair -> eligible for the out projection
                    for t_ in range(qb * (QBS // 128),
                                    (qb + 1) * (QBS // 128)):
                        for cs in range(NCS):
                            oq.append((t_, cs))

                hp = HP - 1
                attention_pair(hp, cur[0], cur[1], pump7, finish_qb,
                               prev_tail=ptail, defer_tail=False)

                while oq:
                    emit_oproj()

    nc.compile()
    return nc


# ---------------------------------------------------------------------------
# host-side sharding
# ---------------------------------------------------------------------------

def _planar_perm():
    """Within-head column permutation: even dims -> 0..31, odd -> 32..63."""
    p = np.empty(HEAD_DIM, dtype=np.int64)
    p[:32] = 2 * np.arange(32)
    p[32:] = 2 * np.arange(32) + 1
    return p


def _rope_tables(T):
    theta = 1.0 / (10000.0 ** (np.arange(0, HEAD_DIM, 2, dtype=np.float64)
                               / HEAD_DIM))  # [32]
    idx = np.outer(np.arange(T, dtype=np.float64), theta)  # [T, 32]
    cos = np.cos(idx).astype(np.float32)
    sin = np.sin(idx).astype(np.float32)
    cosf = np.empty((128, T), dtype=np.float32)
    sinf = np.empty((128, T), dtype=np.float32)
    for r in range(128):
        i = r % 32
        lo = ((r // 32) % 2 == 0)
        cosf[r] = cos[:, i]
        sinf[r] = (-sin[:, i]) if lo else sin[:, i]
    # device reads the sin table at the *source* rows of the pair swap
    # (row r holds the value destined for row r^32), so swap 32-row blocks
    sinf = sinf.reshape(4, 32, T)[[1, 0, 3, 2]].reshape(128, T)
    return cosf, sinf


def make_in_maps(x, W_qkv, b_qkv, W_proj, T, C, HLOC, qk_bias, v_bias):
    B = x.shape[0]
    D = HEAD_DIM
    HP = HLOC // 2
    CL = HLOC * D
    KC = C // 128
    DC = CL // 128
    NTS = T // 512
    VN = min(512, CL)
    NVS = CL // VN
    NGRP = (C // D) // HLOC  # head groups

    Wq = np.asarray(W_qkv[:, 0:C], dtype=np.float32)
    Wk = np.asarray(W_qkv[:, C:2 * C], dtype=np.float32)
    Wv = np.asarray(W_qkv[:, 2 * C:3 * C], dtype=np.float32)
    bq = np.asarray(b_qkv[0:C], dtype=np.float32)
    bk = np.asarray(b_qkv[C:2 * C], dtype=np.float32)
    bv_ = np.asarray(b_qkv[2 * C:3 * C], dtype=np.float32)

    perm = _planar_perm()
    cosf, sinf = _rope_tables(T)
    tri = (np.arange(128)[:, None] <= np.arange(128)[None, :])

    def to_bf(a):
        return np.ascontiguousarray(a.astype(NPBF16))

    grp = {}
    for g in range(NGRP):
        cols_qk = np.concatenate(
            [(g * HLOC + h) * D + perm for h in range(HLOC)])
        cols_v = np.concatenate(
            [(g * HLOC + h) * D + np.arange(D) for h in range(HLOC)])
        wq_g = Wq[:, cols_qk]   # [C, CL]
        wk_g = Wk[:, cols_qk]
        wv_g = Wv[:, cols_v]
        wo_g = np.asarray(W_proj[g * CL:(g + 1) * CL, :], dtype=np.float32)

        ent = {
            "wq": to_bf(wq_g.reshape(KC, 128, HP, 128)
                        .transpose(2, 1, 0, 3).reshape(HP, 128, KC * 128)),
            "wk": to_bf(wk_g.reshape(KC, 128, HP, 128)
                        .transpose(2, 1, 0, 3).reshape(HP, 128, KC * 128)),
            # [NVS, 128, KC*VN]: half h holds column-slice h of every k-tile
            "wv": to_bf(wv_g.reshape(KC, 128, NVS, VN)
                        .transpose(2, 1, 0, 3).reshape(NVS, 128, KC * VN)),
            # [128, DC*C]: d-major packing of the DC row-tiles
            "wo": to_bf(wo_g.reshape(DC, 128, C)
                        .transpose(1, 0, 2).reshape(128, DC * C)),
            "cosf": to_bf(cosf),
            "sinf": to_bf(sinf),
            "tri": to_bf(tri.astype(np.float32)),
        }
        if qk_bias:
            bqk_t = np.empty((128, 2 * HP), dtype=np.float32)
            bq_g = bq[cols_qk]
            bk_g = bk[cols_qk]
            for hp in range(HP):
                bqk_t[:, hp] = bq_g[hp * 128:(hp + 1) * 128]
                bqk_t[:, HP + hp] = bk_g[hp * 128:(hp + 1) * 128]
            ent["bqk"] = bqk_t
        if v_bias:
            bv_t = np.empty((128, HP), dtype=np.float32)
            bv_g = bv_[cols_v]
            for hp in range(HP):
                bv_t[:, hp] = bv_g[hp * 128:(hp + 1) * 128]
            ent["bv"] = bv_t
        grp[g] = ent

    in_maps = []
    for core in range(B * NGRP):
        b, g = core // NGRP, core % NGRP
        m = dict(grp[g])
        # x^T packed chunk-major: [128, (ts, k, 512)]
        xtT = np.asarray(x[b], dtype=np.float32).T  # [C, T]
        m["xt"] = to_bf(xtT.reshape(KC, 128, NTS, 512)
                        .transpose(1, 2, 0, 3).reshape(128, NTS * KC * 512))
        in_maps.append(m)
    return in_maps


_CACHE = {}


def _get_graph(T, C, HLOC, qk_bias, v_bias):
    key = (T, C, HLOC, qk_bias, v_bias)
    if key not in _CACHE:
        _CACHE[key] = build_graph(T, C, HLOC, qk_bias, v_bias)
    return _CACHE[key]


def _ensure_ntff_hook():
    """Register the axon NTFF profile hook if the image's antenv lacks it."""
    import sys
    import types
    import antenv
    try:
        from antenv import axon_hooks  # noqa: F401
    except ImportError:
        mod = types.ModuleType("antenv.axon_hooks")
        mod._hook = None

        def set_axon_ntff_profile_hook(h, _m=mod):
            _m._hook = h

        def get_axon_ntff_profile_hook(_m=mod):
            return _m._hook

        mod.set_axon_ntff_profile_hook = set_axon_ntff_profile_hook
        mod.get_axon_ntff_profile_hook = get_axon_ntff_profile_hook
        sys.modules["antenv.axon_hooks"] = mod
        antenv.axon_hooks = mod
    from antenv.axon_hooks import (get_axon_ntff_profile_hook,
                                   set_axon_ntff_profile_hook)
    if get_axon_ntff_profile_hook() is None:
        from trn_agent_boot.trn_boot import _ntff_profile_via_ctypes
        set_axon_ntff_profile_hook(
            _ntff_profile_via_ctypes("/opt/axon/libaxon_pjrt.so"))


def run(inputs, trace=False):
    from concourse.bass_utils import run_bass_kernel_spmd
    if trace:
        try:
            _ensure_ntff_hook()
        except Exception as e:
            print(f"ntff hook setup failed: {e}")
    x = np.asarray(inputs["x"])
    W_qkv = np.asarray(inputs["W_qkv"])
    b_qkv = np.asarray(inputs["b_qkv"])
    W_proj = np.asarray(inputs["W_proj"])
    b_proj = np.asarray(inputs["b_proj"])
    B, T, C = x.shape
    HLOC = HLOC_FULL
    NGRP = (C // HEAD_DIM) // HLOC

    qk_bias = bool(np.any(b_qkv[0:2 * C]))
    v_bias = bool(np.any(b_qkv[2 * C:]))
    nc = _get_graph(T, C, HLOC, qk_bias, v_bias)
    in_maps = make_in_maps(x, W_qkv, b_qkv, W_proj, T, C, HLOC,
                           qk_bias, v_bias)
    res = run_bass_kernel_spmd(nc, in_maps, core_ids=list(range(len(in_maps))),
                               trace=trace)
    out = np.empty((B, T, C), dtype=np.float32)
    for b in range(B):
        acc = None
        for g in range(NGRP):
            part = np.asarray(res.results[b * NGRP + g]["out"],
                              dtype=np.float32)
            acc = part if acc is None else acc + part
        out[b] = acc + b_proj[None, :].astype(np.float32)
    return out, res


def kernel(**inputs):
    out, _ = run(inputs, trace=False)
    return out
